# revision 1
# baseline (speedup 1.0000x reference)
"""Grouped MLP (MoE expert FFN) Bass kernel for 8 Trainium2 NeuronCores.

Problem: 4096 tokens sorted by expert (8 experts, uneven counts), per-expert
GLU MLP:  h = x @ w1[g]  (-> up|gate, 2*2048 cols);  a = silu(up)*gate;
y = a @ w2[g].

Sharding: expert-parallel.  Core g handles expert g's tokens (padded to a
common Tpad so all 8 cores run one identical program).  All shard/gather work
happens on the host; there are no device collectives.

Device program (per core), everything in transposed "feature-major" space so
weights are the stationary matmul operand with natural layouts:
  h^T[mi]  = sum_kc w1[kc, mi]^T @ x^T[kc]      (PSUM accum over K=1024)
  hgl[mi]  = silu(up) * gate                    (ACT + DVE, PSUM->SBUF)
  y^T[mo]  = sum_ki w2[ki, mo]^T @ hgl[ki]      (PSUM accum over K=2048)
Matmuls run as float32r (full PE rate at free-dim >= 256) on fp32 data.
"""

import sys

try:  # concourse normally comes from the container's PYTHONPATH
    import concourse  # noqa: F401
except ImportError:  # pragma: no cover - fallback for stripped env
    for _p in (
        "/root/.axon_site",
        "/root/.axon_site/_ro/trn_rl_repo",
        "/root/.axon_site/_ro/pypackages",
        "/opt/trn_rl_repo",
    ):
        if _p not in sys.path:
            sys.path.append(_p)

from contextlib import ExitStack

import numpy as np

NUM_TOKENS = 4096
HIDDEN = 1024
INTER = 2048
GROUPS = 8
N_CORES = 8

F32 = None  # set lazily after imports


def _ceil_to(x: int, m: int) -> int:
    return ((x + m - 1) // m) * m


_PROGRAM_CACHE: dict = {}


def _build_program(tpad: int):
    """Build + compile the single-core Bass program (same NEFF on all cores)."""
    import concourse.bass as bass  # noqa: F401
    import concourse.mybir as mybir
    import concourse.tile as tile
    from concourse import bacc

    f32 = mybir.dt.float32
    f32r = mybir.dt.float32r
    silu = mybir.ActivationFunctionType.Silu

    KC = HIDDEN // 128          # 8  k-blocks for fc1
    MI = INTER // 128           # 16 output row-blocks of h (pairs up+gate)
    KI = INTER // 128           # 16 k-blocks for fc2
    MO = HIDDEN // 128          # 8  output row-blocks of y

    # token chunks (free dim per matmul; <=512 for 4-byte dtypes)
    nts = []
    off = 0
    while off < tpad:
        nl = min(512, tpad - off)
        nts.append((off, nl))
        off += nl

    nc = bacc.Bacc("TRN2", target_bir_lowering=False, debug=False)

    xT_d = nc.dram_tensor("xT", [KC, 128, tpad], f32r, kind="ExternalInput").ap()
    w1_d = nc.dram_tensor("w1c", [MI, KC, 128, 256], f32r, kind="ExternalInput").ap()
    w2_d = nc.dram_tensor("w2c", [MO, KI, 128, 128], f32r, kind="ExternalInput").ap()
    y_d = nc.dram_tensor("yT", [MO, 128, tpad], f32, kind="ExternalOutput").ap()

    with tile.TileContext(nc) as tc, ExitStack() as ctx:
        xp = ctx.enter_context(tc.tile_pool(name="x", bufs=1))
        hp = ctx.enter_context(tc.tile_pool(name="hgl", bufs=1))
        yp = ctx.enter_context(tc.tile_pool(name="y", bufs=1))
        w1p = ctx.enter_context(tc.tile_pool(name="w1", bufs=3))
        w2p = ctx.enter_context(tc.tile_pool(name="w2", bufs=3))
        pup = ctx.enter_context(tc.tile_pool(name="pu", bufs=2, space="PSUM"))
        pgp = ctx.enter_context(tc.tile_pool(name="pg", bufs=2, space="PSUM"))
        pyp = ctx.enter_context(tc.tile_pool(name="py", bufs=2, space="PSUM"))
        tp = ctx.enter_context(tc.tile_pool(name="tmp", bufs=3))

        x_sb = xp.tile([128, KC * tpad], f32r)
        for kc in range(KC):
            nc.sync.dma_start(
                out=x_sb[:, kc * tpad : (kc + 1) * tpad], in_=xT_d[kc]
            )

        hgl = hp.tile([128, KI * tpad], f32r)
        y_sb = yp.tile([128, MO * tpad], f32)

        # ---- fc1 + GLU ----
        for mi in range(MI):
            w1t = w1p.tile([128, KC * 256], f32r)
            for kc in range(KC):
                nc.sync.dma_start(
                    out=w1t[:, kc * 256 : (kc + 1) * 256], in_=w1_d[mi, kc]
                )
            for no, nl in nts:
                pu = pup.tile([128, 512], f32)
                pg = pgp.tile([128, 512], f32)
                for kc in range(KC):
                    rhs = x_sb[:, kc * tpad + no : kc * tpad + no + nl]
                    nc.tensor.matmul(
                        pu[:, :nl],
                        w1t[:, kc * 256 : kc * 256 + 128],
                        rhs,
                        start=(kc == 0),
                        stop=(kc == KC - 1),
                    )
                    nc.tensor.matmul(
                        pg[:, :nl],
                        w1t[:, kc * 256 + 128 : kc * 256 + 256],
                        rhs,
                        start=(kc == 0),
                        stop=(kc == KC - 1),
                    )
                tmp = tp.tile([128, 512], f32)
                nc.scalar.activation(tmp[:, :nl], pu[:, :nl], silu)
                nc.vector.tensor_mul(
                    hgl[:, mi * tpad + no : mi * tpad + no + nl],
                    tmp[:, :nl],
                    pg[:, :nl],
                )

        # ---- fc2 ----
        for mo in range(MO):
            w2t = w2p.tile([128, KI * 128], f32r)
            for ki in range(KI):
                nc.sync.dma_start(
                    out=w2t[:, ki * 128 : (ki + 1) * 128], in_=w2_d[mo, ki]
                )
            for no, nl in nts:
                py = pyp.tile([128, 512], f32)
                for ki in range(KI):
                    nc.tensor.matmul(
                        py[:, :nl],
                        w2t[:, ki * 128 : ki * 128 + 128],
                        hgl[:, ki * tpad + no : ki * tpad + no + nl],
                        start=(ki == 0),
                        stop=(ki == KI - 1),
                    )
                nc.scalar.copy(
                    y_sb[:, mo * tpad + no : mo * tpad + no + nl], py[:, :nl]
                )

        for mo in range(MO):
            nc.sync.dma_start(
                out=y_d[mo], in_=y_sb[:, mo * tpad : (mo + 1) * tpad]
            )

    nc.compile()
    return nc


def _get_program(tpad: int):
    if tpad not in _PROGRAM_CACHE:
        _PROGRAM_CACHE[tpad] = _build_program(tpad)
    return _PROGRAM_CACHE[tpad]


def _prep_core_inputs(x_seg: np.ndarray, w1_g: np.ndarray, w2_g: np.ndarray, tpad: int):
    """Host-side shard prep for one core: transpose/pad tokens, retile weights."""
    cnt = x_seg.shape[0]
    xT = np.zeros((HIDDEN, tpad), np.float32)
    if cnt:
        xT[:, :cnt] = x_seg.T
    xT = np.ascontiguousarray(xT.reshape(HIDDEN // 128, 128, tpad))

    # w1_g: [1024, 4096] cols = up[0:2048] | gate[2048:4096]
    # -> [mi 16, kc 8, 128, 256] where cols 0:128 = up(mi), 128:256 = gate(mi)
    w1c = np.ascontiguousarray(
        w1_g.reshape(8, 128, 2, 16, 128).transpose(3, 0, 1, 2, 4).reshape(16, 8, 128, 256)
    )
    # w2_g: [2048, 1024] -> [mo 8, ki 16, 128, 128]
    w2c = np.ascontiguousarray(
        w2_g.reshape(16, 128, 8, 128).transpose(2, 0, 1, 3)
    )
    return {"xT": xT, "w1c": w1c, "w2c": w2c}


_LAST_RESULTS = {}  # exposed for test.py (exec time, trace paths)


def kernel(permuted_tokens, tokens_per_expert, w1, w2, _trace=False):
    from concourse.bass_utils import run_bass_kernel_spmd

    x = np.asarray(permuted_tokens, np.float32)
    counts = np.asarray(tokens_per_expert, np.int64)
    w1 = np.asarray(w1, np.float32)
    w2 = np.asarray(w2, np.float32)

    offs = np.zeros(GROUPS + 1, np.int64)
    offs[1:] = np.cumsum(counts)
    tpad = max(256, _ceil_to(int(counts.max()), 128))

    nc = _get_program(tpad)

    in_maps = []
    for g in range(GROUPS):
        in_maps.append(
            _prep_core_inputs(x[offs[g] : offs[g + 1]], w1[g], w2[g], tpad)
        )

    kwargs = {}
    if _trace:
        kwargs = dict(trace=True, trace_cores=list(range(N_CORES)))
    res = run_bass_kernel_spmd(nc, in_maps, core_ids=list(range(N_CORES)), **kwargs)
    _LAST_RESULTS["res"] = res

    out = np.empty((x.shape[0], HIDDEN), np.float32)
    for g in range(GROUPS):
        cnt = int(counts[g])
        if cnt == 0:
            continue
        yT = res.results[g]["yT"].reshape(HIDDEN, tpad)
        out[offs[g] : offs[g + 1]] = yT[:, :cnt].T
    return out



# revision 2
# speedup vs baseline: 1.7575x; 1.7575x over previous
"""Grouped MLP (MoE expert FFN) Bass kernel for 8 Trainium2 NeuronCores.

Problem: 4096 tokens sorted by expert (8 experts, uneven counts), per-expert
GLU MLP:  h = x @ w1[g]  (-> up|gate, 2*2048 cols);  a = silu(up)*gate;
y = a @ w2[g].

Sharding: tensor-parallel over the INTER dim.  Core c owns a 256-wide slice
of INTER for ALL experts: fc1 column-slice (256 up cols + 256 gate cols per
expert), fc2 row-slice (256 rows per expert).  Every core processes every
token, so per-core work is 512-token-equivalent regardless of the expert
token distribution (perfect load balance), and each weight byte lands on
exactly one core.  Partial fc2 outputs are summed on the host.

Device program (per core), bf16 matmuls accumulated in fp32 PSUM, in
"feature-major" (transposed) space; tokens are processed in chunks of <=512
(one chunk belongs to one expert):
  hT[j]  = sum_k w1s[k,j]^T @ xT[k]       j in {up0,up1,gt0,gt1}
  hglT   = silu(up_i) * gate_i            (ACT + DVE, PSUM->SBUF, bf16)
  yT[hb] = sum_ki w2s[ki,hb]^T @ hglT[ki] (2-term accum), cast bf16, DMA out

All DRAM<->SBUF transfers are laid out host-side so each DMA moves
[128 partitions x multi-KB contiguous lines] (no fragmented descriptors).
fc2 of chunk i is emitted after fc1 of chunk i+1 (software skew) so the
PE never waits on the GLU of the chunk it just produced.
"""

import sys

try:  # concourse normally comes from the container's PYTHONPATH
    import concourse  # noqa: F401
except ImportError:  # pragma: no cover - fallback for stripped env
    for _p in (
        "/root/.axon_site",
        "/root/.axon_site/_ro/trn_rl_repo",
        "/root/.axon_site/_ro/pypackages",
        "/opt/trn_rl_repo",
    ):
        if _p not in sys.path:
            sys.path.append(_p)

from contextlib import ExitStack

import numpy as np
import ml_dtypes

BF16 = np.dtype(ml_dtypes.bfloat16)

NUM_TOKENS = 4096
HIDDEN = 1024
INTER = 2048
GROUPS = 8
N_CORES = 8

SLICE = INTER // N_CORES       # 256 inter cols/rows per core
CHUNK = 512                    # max tokens per chunk (PSUM fp32 free-dim cap)
KC = HIDDEN // 128             # 8 contraction blocks for fc1
W1_COLS = 4 * 128 * KC         # 4096: per-k [up0, up1, gt0, gt1] x 128
W2_COLS = 2 * HIDDEN           # 2048: per-ki 1024 hid cols
WC_COLS = W1_COLS + W2_COLS    # 6144


def _chunks_from_counts(counts):
    """Split each expert's token range into near-equal chunks of <= CHUNK."""
    chunks = []  # (expert, token_offset, n)
    off = 0
    for g in range(GROUPS):
        cnt = int(counts[g])
        if cnt <= 0:
            continue
        parts = -(-cnt // CHUNK)
        base, rem = divmod(cnt, parts)
        for i in range(parts):
            n = base + (1 if i < rem else 0)
            chunks.append((g, off, n))
            off += n
    return chunks


_PROGRAM_CACHE: dict = {}


def _build_program(key):
    """Build + compile the single-core Bass program (same NEFF on all cores).

    key = tuple of (expert, n_tokens) per chunk, in token order.
    """
    import concourse.bass as bass  # noqa: F401
    import concourse.mybir as mybir
    import concourse.tile as tile
    from concourse import bacc

    f32 = mybir.dt.float32
    bf16 = mybir.dt.bfloat16
    silu = mybir.ActivationFunctionType.Silu

    T = sum(n for _, n in key)

    nc = bacc.Bacc("TRN2", target_bir_lowering=False, debug=False)

    x_d = nc.dram_tensor("xc", [128, KC * T], bf16, kind="ExternalInput").ap()
    w_d = nc.dram_tensor("wc", [GROUPS, 128, WC_COLS], bf16, kind="ExternalInput").ap()
    y_d = nc.dram_tensor("yc", [128, 8 * T], bf16, kind="ExternalOutput").ap()

    with tile.TileContext(nc) as tc, ExitStack() as ctx:
        xp = ctx.enter_context(tc.tile_pool(name="x", bufs=4))
        wp = ctx.enter_context(tc.tile_pool(name="w", bufs=4))
        hp = ctx.enter_context(tc.tile_pool(name="hgl", bufs=3))
        yp = ctx.enter_context(tc.tile_pool(name="y", bufs=3))
        tp = ctx.enter_context(tc.tile_pool(name="tmp", bufs=4))
        p1 = ctx.enter_context(tc.tile_pool(name="p1", bufs=5, space="PSUM"))
        p2 = ctx.enter_context(tc.tile_pool(name="p2", bufs=3, space="PSUM"))

        wt = {}  # expert -> SBUF weight tile

        def emit_fc2(g, hgl, off, n):
            w = wt[g]
            y_sb = yp.tile([128, 8 * n], bf16, tag="y")
            for hb in range(8):
                py = p2.tile([128, n], f32, tag="p2")
                nc.tensor.matmul(
                    py,
                    w[:, W1_COLS + hb * 128 : W1_COLS + hb * 128 + 128],
                    hgl[:, :n],
                    start=True,
                    stop=False,
                )
                nc.tensor.matmul(
                    py,
                    w[:, W1_COLS + HIDDEN + hb * 128 : W1_COLS + HIDDEN + hb * 128 + 128],
                    hgl[:, n : 2 * n],
                    start=False,
                    stop=True,
                )
                dst = y_sb[:, hb * n : (hb + 1) * n]
                if hb % 2 == 0:
                    nc.scalar.copy(dst, py)
                else:
                    nc.vector.tensor_copy(dst, py)
            nc.sync.dma_start(out=y_d[:, 8 * off : 8 * (off + n)], in_=y_sb)

        pending = None  # (expert, hgl tile, token_offset, n) awaiting fc2
        off = 0
        for g, n in key:
            if g not in wt:
                w = wp.tile([128, WC_COLS], bf16, tag="w")
                nc.sync.dma_start(out=w, in_=w_d[g])
                wt[g] = w
            xt = xp.tile([128, KC * n], bf16, tag="x")
            nc.sync.dma_start(out=xt, in_=x_d[:, KC * off : KC * (off + n)])

            # fc1: j in {0: up0, 1: up1, 2: gt0, 3: gt1}; order pairs (up,gt)
            ps = {}
            for j in (0, 2, 1, 3):
                p = p1.tile([128, n], f32, tag="p1")
                for k in range(KC):
                    nc.tensor.matmul(
                        p,
                        wt[g][:, k * 512 + j * 128 : k * 512 + j * 128 + 128],
                        xt[:, k * n : (k + 1) * n],
                        start=(k == 0),
                        stop=(k == KC - 1),
                    )
                ps[j] = p

            hgl = hp.tile([128, 2 * n], bf16, tag="h")
            for ib in range(2):
                tmp = tp.tile([128, n], f32, tag="t")
                nc.scalar.activation(tmp, ps[ib], silu)
                nc.vector.tensor_mul(hgl[:, ib * n : (ib + 1) * n], tmp, ps[2 + ib])

            if pending is not None:
                emit_fc2(*pending)
            pending = (g, hgl, off, n)
            off += n

        emit_fc2(*pending)

    nc.compile()
    return nc


def _get_program(key):
    if key not in _PROGRAM_CACHE:
        _PROGRAM_CACHE[key] = _build_program(key)
    return _PROGRAM_CACHE[key]


def _prep_x(x, chunks, T):
    """[T, 1024] fp32 -> [128, 8*T] bf16, chunk-major k-blocked layout."""
    xb = x.astype(BF16)
    X = np.empty((128, KC * T), BF16)
    for _, off, n in chunks:
        seg = xb[off : off + n].T  # [1024, n]
        X[:, KC * off : KC * (off + n)] = (
            seg.reshape(KC, 128, n).transpose(1, 0, 2).reshape(128, KC * n)
        )
    return X


def _prep_weights(w1b, w2b, c):
    """Per-core slices of w1/w2 (already bf16) -> [8, 128, 6144]."""
    wc = np.empty((GROUPS, 128, WC_COLS), BF16)
    lo, hi = c * SLICE, (c + 1) * SLICE
    for g in range(GROUPS):
        sl = np.concatenate([w1b[g][:, lo:hi], w1b[g][:, INTER + lo : INTER + hi]], 1)
        wc[g, :, :W1_COLS] = (
            sl.reshape(KC, 128, 2 * SLICE).transpose(1, 0, 2).reshape(128, W1_COLS)
        )
        w2s = w2b[g][lo:hi]  # [256, 1024]
        wc[g, :, W1_COLS:] = (
            w2s.reshape(2, 128, HIDDEN).transpose(1, 0, 2).reshape(128, W2_COLS)
        )
    return wc


_LAST_RESULTS = {}  # exposed for test.py (exec time, trace paths)


def kernel(permuted_tokens, tokens_per_expert, w1, w2, _trace=False):
    from concourse.bass_utils import run_bass_kernel_spmd

    x = np.asarray(permuted_tokens, np.float32)
    counts = np.asarray(tokens_per_expert, np.int64)
    w1 = np.asarray(w1, np.float32)
    w2 = np.asarray(w2, np.float32)

    chunks = _chunks_from_counts(counts)
    T = sum(n for _, _, n in chunks)
    key = tuple((g, n) for g, _, n in chunks)

    nc = _get_program(key)

    X = _prep_x(x, chunks, T)
    w1b = w1.astype(BF16)
    w2b = w2.astype(BF16)
    in_maps = [{"xc": X, "wc": _prep_weights(w1b, w2b, c)} for c in range(N_CORES)]

    kwargs = {}
    if _trace:
        kwargs = dict(trace=True, trace_cores=list(range(N_CORES)))
    res = run_bass_kernel_spmd(nc, in_maps, core_ids=list(range(N_CORES)), **kwargs)
    _LAST_RESULTS["res"] = res

    acc = np.zeros((128, 8 * T), np.float32)
    for c in range(N_CORES):
        acc += np.asarray(res.results[c]["yc"]).astype(np.float32)

    out = np.zeros((x.shape[0], HIDDEN), np.float32)
    for _, off, n in chunks:
        seg = acc[:, 8 * off : 8 * (off + n)].reshape(128, 8, n)
        out[off : off + n] = seg.transpose(2, 1, 0).reshape(n, HIDDEN)
    return out


# revision 3
# speedup vs baseline: 1.8557x; 1.0559x over previous
"""Grouped MLP (MoE expert FFN) Bass kernel for 8 Trainium2 NeuronCores.

Problem: 4096 tokens sorted by expert (8 experts, uneven counts), per-expert
GLU MLP:  h = x @ w1[g]  (-> up|gate, 2*2048 cols);  a = silu(up)*gate;
y = a @ w2[g].

Sharding: tensor-parallel over the INTER dim.  Core c owns a 256-wide slice
of INTER for ALL experts: fc1 column-slice (256 up cols + 256 gate cols per
expert), fc2 row-slice (256 rows per expert).  Every core processes every
token, so per-core work is 512-token-equivalent regardless of the expert
token distribution (perfect load balance), and each weight byte lands on
exactly one core.  Partial fc2 outputs are summed on the host.

Device program (per core), bf16 matmuls accumulated in fp32 PSUM, in
"feature-major" (transposed) space; tokens are processed in chunks of <=512
(one chunk belongs to one expert):
  hT[p]  = sum_k w1s[k,p]^T @ xT[k]       p in {pair0, pair1} x {up, gt}
  hglT   = silu(up_p) * gate_p            (ACT + DVE, PSUM->SBUF, bf16)
  yT[hb] = sum_ki w2s[ki,hb]^T @ hglT[ki] (2-term accum), cast bf16, DMA out

All DRAM<->SBUF transfers are laid out host-side so each DMA moves
[128 partitions x multi-KB contiguous lines].  Weights stream in three
0.5MB pieces per expert (up|gt pair 0, pair 1, w2) with the w2 piece
deferred one chunk, so the first matmul only waits on ~1MB of DMA.
fc2 of chunk i is emitted after fc1 of chunk i+1 (software skew) so the
PE never waits on the GLU of the chunk it just produced.  The last chunk
is capped at 128 tokens to shrink the kernel tail.
"""

import sys

try:  # concourse normally comes from the container's PYTHONPATH
    import concourse  # noqa: F401
except ImportError:  # pragma: no cover - fallback for stripped env
    for _p in (
        "/root/.axon_site",
        "/root/.axon_site/_ro/trn_rl_repo",
        "/root/.axon_site/_ro/pypackages",
        "/opt/trn_rl_repo",
    ):
        if _p not in sys.path:
            sys.path.append(_p)

from contextlib import ExitStack

import numpy as np
import ml_dtypes

BF16 = np.dtype(ml_dtypes.bfloat16)

NUM_TOKENS = 4096
HIDDEN = 1024
INTER = 2048
GROUPS = 8
N_CORES = 8

SLICE = INTER // N_CORES       # 256 inter cols/rows per core
CHUNK = 512                    # max tokens per chunk (PSUM fp32 free-dim cap)
KC = HIDDEN // 128             # 8 contraction blocks for fc1
PAIR_COLS = KC * 256           # 2048 cols per up|gt pair piece
W1_COLS = 2 * PAIR_COLS        # 4096
W2_COLS = 2 * HIDDEN           # 2048: per-ki 1024 hid cols
WC_COLS = W1_COLS + W2_COLS    # 6144


def _chunks_from_counts(counts):
    """Split each expert's token range into near-equal chunks of <= CHUNK."""
    chunks = []  # (expert, token_offset, n)
    off = 0
    for g in range(GROUPS):
        cnt = int(counts[g])
        if cnt <= 0:
            continue
        parts = -(-cnt // CHUNK)
        base, rem = divmod(cnt, parts)
        for i in range(parts):
            n = base + (1 if i < rem else 0)
            chunks.append((g, off, n))
            off += n
    # small last chunk -> short kernel tail (fc2+store of the final chunk
    # cannot overlap anything)
    if chunks and chunks[-1][2] > 256:
        g, off, n = chunks[-1]
        chunks[-1] = (g, off, n - 128)
        chunks.append((g, off + n - 128, 128))
    return chunks


_PROGRAM_CACHE: dict = {}


def _build_program(key):
    """Build + compile the single-core Bass program (same NEFF on all cores).

    key = tuple of (expert, n_tokens) per chunk, in token order.
    """
    import concourse.bass as bass  # noqa: F401
    import concourse.mybir as mybir
    import concourse.tile as tile
    from concourse import bacc

    f32 = mybir.dt.float32
    bf16 = mybir.dt.bfloat16
    silu = mybir.ActivationFunctionType.Silu

    T = sum(n for _, n in key)

    nc = bacc.Bacc("TRN2", target_bir_lowering=False, debug=False)

    x_d = nc.dram_tensor("xc", [128, KC * T], bf16, kind="ExternalInput").ap()
    w_d = nc.dram_tensor("wc", [GROUPS, 128, WC_COLS], bf16, kind="ExternalInput").ap()
    y_d = nc.dram_tensor("yc", [128, 8 * T], bf16, kind="ExternalOutput").ap()

    with tile.TileContext(nc) as tc, ExitStack() as ctx:
        xp = ctx.enter_context(tc.tile_pool(name="x", bufs=4))
        wp = ctx.enter_context(tc.tile_pool(name="w", bufs=4))
        hp = ctx.enter_context(tc.tile_pool(name="hgl", bufs=3))
        yp = ctx.enter_context(tc.tile_pool(name="y", bufs=3))
        tp = ctx.enter_context(tc.tile_pool(name="tmp", bufs=4))
        p1 = ctx.enter_context(tc.tile_pool(name="p1", bufs=5, space="PSUM"))
        p2 = ctx.enter_context(tc.tile_pool(name="p2", bufs=3, space="PSUM"))

        wt = {}          # expert -> SBUF weight tile
        p2_pending = []  # experts whose w2 piece DMA is deferred

        def flush_p2():
            while p2_pending:
                g = p2_pending.pop(0)
                nc.sync.dma_start(
                    out=wt[g][:, W1_COLS:WC_COLS], in_=w_d[g][:, W1_COLS:WC_COLS]
                )

        def emit_fc2(g, hgl, off, n):
            w = wt[g]
            y_sb = yp.tile([128, 8 * n], bf16, tag="y")
            for hb in range(8):
                py = p2.tile([128, n], f32, tag="p2")
                nc.tensor.matmul(
                    py,
                    w[:, W1_COLS + hb * 128 : W1_COLS + hb * 128 + 128],
                    hgl[:, :n],
                    start=True,
                    stop=False,
                )
                nc.tensor.matmul(
                    py,
                    w[:, W1_COLS + HIDDEN + hb * 128 : W1_COLS + HIDDEN + hb * 128 + 128],
                    hgl[:, n : 2 * n],
                    start=False,
                    stop=True,
                )
                dst = y_sb[:, hb * n : (hb + 1) * n]
                if hb % 2 == 0:
                    nc.scalar.copy(dst, py)
                else:
                    nc.vector.tensor_copy(dst, py)
            nc.sync.dma_start(out=y_d[:, 8 * off : 8 * (off + n)], in_=y_sb)

        pending = None  # (expert, hgl tile, token_offset, n) awaiting fc2
        off = 0
        for g, n in key:
            xt = xp.tile([128, KC * n], bf16, tag="x")
            nc.sync.dma_start(out=xt, in_=x_d[:, KC * off : KC * (off + n)])
            if g not in wt:
                w = wp.tile([128, WC_COLS], bf16, tag="w")
                nc.sync.dma_start(out=w[:, :PAIR_COLS], in_=w_d[g][:, :PAIR_COLS])
                nc.sync.dma_start(
                    out=w[:, PAIR_COLS:W1_COLS], in_=w_d[g][:, PAIR_COLS:W1_COLS]
                )
                wt[g] = w
                deferred = True
            else:
                deferred = False

            # fc1: pair p in {0, 1}; piece p holds k-major [up_p | gt_p]
            ps = {}
            for p in (0, 1):
                for half in (0, 1):  # 0: up, 1: gate
                    acc = p1.tile([128, n], f32, tag="p1")
                    for k in range(KC):
                        base = p * PAIR_COLS + k * 256 + half * 128
                        nc.tensor.matmul(
                            acc,
                            wt[g][:, base : base + 128],
                            xt[:, k * n : (k + 1) * n],
                            start=(k == 0),
                            stop=(k == KC - 1),
                        )
                    ps[(p, half)] = acc

            hgl = hp.tile([128, 2 * n], bf16, tag="h")
            for p in range(2):
                tmp = tp.tile([128, n], f32, tag="t")
                nc.scalar.activation(tmp, ps[(p, 0)], silu)
                nc.vector.tensor_mul(hgl[:, p * n : (p + 1) * n], tmp, ps[(p, 1)])

            flush_p2()
            if deferred:
                p2_pending.append(g)
            if pending is not None:
                emit_fc2(*pending)
            pending = (g, hgl, off, n)
            off += n

        flush_p2()
        emit_fc2(*pending)

    nc.compile()
    return nc


def _get_program(key):
    if key not in _PROGRAM_CACHE:
        _PROGRAM_CACHE[key] = _build_program(key)
    return _PROGRAM_CACHE[key]


def _prep_x(x, chunks, T):
    """[T, 1024] fp32 -> [128, 8*T] bf16, chunk-major k-blocked layout."""
    xb = x.astype(BF16)
    X = np.empty((128, KC * T), BF16)
    for _, off, n in chunks:
        seg = xb[off : off + n].T  # [1024, n]
        X[:, KC * off : KC * (off + n)] = (
            seg.reshape(KC, 128, n).transpose(1, 0, 2).reshape(128, KC * n)
        )
    return X


def _prep_weights(w1b, w2b, c):
    """Per-core slices of w1/w2 (already bf16) -> [8, 128, 6144].

    cols [p*2048 + k*256 + half*128 : +128] = w1 block (pair p, k, up/gt)
    cols [4096 + ki*1024 : +1024]           = w2 block ki
    """
    wc = np.empty((GROUPS, 128, WC_COLS), BF16)
    lo = c * SLICE
    for g in range(GROUPS):
        for p in range(2):
            u = w1b[g][:, lo + p * 128 : lo + p * 128 + 128]
            gt = w1b[g][:, INTER + lo + p * 128 : INTER + lo + p * 128 + 128]
            sl = np.concatenate([u, gt], 1)  # [1024, 256]
            wc[g, :, p * PAIR_COLS : (p + 1) * PAIR_COLS] = (
                sl.reshape(KC, 128, 256).transpose(1, 0, 2).reshape(128, PAIR_COLS)
            )
        w2s = w2b[g][lo : lo + SLICE]  # [256, 1024]
        wc[g, :, W1_COLS:] = (
            w2s.reshape(2, 128, HIDDEN).transpose(1, 0, 2).reshape(128, W2_COLS)
        )
    return wc


_LAST_RESULTS = {}  # exposed for test.py (exec time, trace paths)


def kernel(permuted_tokens, tokens_per_expert, w1, w2, _trace=False):
    from concourse.bass_utils import run_bass_kernel_spmd

    x = np.asarray(permuted_tokens, np.float32)
    counts = np.asarray(tokens_per_expert, np.int64)
    w1 = np.asarray(w1, np.float32)
    w2 = np.asarray(w2, np.float32)

    chunks = _chunks_from_counts(counts)
    T = sum(n for _, _, n in chunks)
    key = tuple((g, n) for g, _, n in chunks)

    nc = _get_program(key)

    X = _prep_x(x, chunks, T)
    w1b = w1.astype(BF16)
    w2b = w2.astype(BF16)
    in_maps = [{"xc": X, "wc": _prep_weights(w1b, w2b, c)} for c in range(N_CORES)]

    kwargs = {}
    if _trace:
        kwargs = dict(trace=True, trace_cores=list(range(N_CORES)))
    res = run_bass_kernel_spmd(nc, in_maps, core_ids=list(range(N_CORES)), **kwargs)
    _LAST_RESULTS["res"] = res

    acc = np.zeros((128, 8 * T), np.float32)
    for c in range(N_CORES):
        acc += np.asarray(res.results[c]["yc"]).astype(np.float32)

    out = np.zeros((x.shape[0], HIDDEN), np.float32)
    for _, off, n in chunks:
        seg = acc[:, 8 * off : 8 * (off + n)].reshape(128, 8, n)
        out[off : off + n] = seg.transpose(2, 1, 0).reshape(n, HIDDEN)
    return out
